# revision 6
# baseline (speedup 1.0000x reference)
"""VQ codebook bottleneck (nn_BottleneckBlock) Trainium2 kernel.

Data-parallel over tokens across 8 NeuronCores. Per core:
  - PE computes q[i,j] = 2*x_i.k_j via fp32 matmuls into PSUM
  - DVE tensor_tensor_reduce fuses (+(-|k_j|^2)), the per-token max, and the
    spill of q' to SBUF into one pass
  - index extraction: (q' == m)*iota summed, alternating DVE/GPSIMD
  - dequant gather via indirect DMA from the codebook + PE transpose
  - scalars (fit/commit/prenorm) from ACT accumulated sums, finished on host
"""

import sys
import time

sys.path.insert(0, "/opt/trn_rl_repo")

import numpy as np

import bass_rust
import concourse.bass as bass
import concourse.tile as tile
from concourse import mybir

# ---------------------------------------------------------------------------
# Patch: this container's walrus caps non-EventSemaphore instructions at one
# sem-wait, but TileContext's tail drain aggregates every proc's wait onto a
# single Drain. Split the overflow waits across a chain of drains.
# ---------------------------------------------------------------------------


def _patched_drain_and_barrier(self, tick_clock, wait_clock):
    nc = self.nc
    drain_inst = nc.sync.drain()
    wait_clock.add_sem_waits(
        drain_inst.ins, bass_rust.ScopedClock({None: tick_clock.global_clock})
    )
    si = drain_inst.ins.sync_info
    waits = list(si.on_wait or [])
    if len(waits) > 1:
        si.on_wait = [waits[0]]
        for w in waits[1:]:
            d2 = nc.sync.drain()
            si2 = d2.ins.sync_info
            if si2 is None:
                d2.ins.sync_info = bass_rust.SyncInfo(on_wait=[w], on_update=[])
            else:
                si2.on_wait = [w]
    nc.all_engine_barrier()
    assert self.sems is not None
    popped = nc._tile_sem_poison_stack.pop()
    assert popped is self._sem_poison
    nc.clear_and_free_semaphores(list(self.sems.allocated().values()))
    nc.all_engine_barrier()


tile.TileContext._drain_and_barrier = _patched_drain_and_barrier


def _split_excess_waits(nc):
    """Walrus in this container accepts at most one sem-wait per instruction.
    Hoist extra waits onto EventSemaphore instructions inserted just before,
    on the same engine (same FIFO ordering, same blocking semantics)."""
    n = 0
    for f in nc.m.functions:
        for b in f.blocks:
            insts = b.instructions
            out = []
            changed = False
            for inst in insts:
                si = inst.sync_info
                waits = list(si.on_wait) if (si and si.on_wait) else []
                if len(waits) > 1:
                    for w in waits[:-1]:
                        ev = mybir.InstEventSemaphore(name=f"wsplit_{n}")
                        n += 1
                        ev.engine = inst.engine
                        ev.sync_info = bass_rust.SyncInfo(on_wait=[w], on_update=[])
                        out.append(ev)
                    si.on_wait = [waits[-1]]
                    changed = True
                out.append(inst)
            if changed:
                b.instructions = out

# ---------------------------------------------------------------------------

F32 = mybir.dt.float32
I32 = mybir.dt.int32
ALU = mybir.AluOpType
ACTF = mybir.ActivationFunctionType

N_CORES = 8
P = 128  # tokens per tile
C = 2048  # codes
W = 64  # embedding width


def build_nc(nb: int, ts: int):
    """Build the per-core Bass module. nb = batches, ts = tokens/core/batch.

    Inputs (per core):
      xs   [nb, W+1, ts]  x shard, with a row of ones appended (augmented K row)
      kg   [C, W]         codebook (gather source)
      kt2  [W+1, C]       rows 0..W-1 = 2*k.T, row W = -|k_j|^2
      iota [P, C]         0..C-1 per partition row
    Outputs (per core):
      xg     [nb*ts, W]   gathered k rows, token-major (host transposes)
      accm   [P, ntiles]  per-token max_j(2x.k_j - |k_j|^2)
      accidx [P, ntiles]  per-token argmax index (as f32)
      accs   [W, ntiles]  per-(width,tile) sum of x^2
      accx   [W, ntiles]  per-(width,tile) sum of x
    """
    assert ts % P == 0
    sub = ts // P  # tiles per batch slab
    ntiles = nb * sub
    assert ntiles <= 512

    nc = bass.Bass("TRN2")

    W1 = W + 1
    xs = nc.dram_tensor("xs", [nb, W1, ts], F32, kind="ExternalInput")
    kg = nc.dram_tensor("kg", [C, W], F32, kind="ExternalInput")
    kt2 = nc.dram_tensor("kt2", [W1, C], F32, kind="ExternalInput")
    iota = nc.dram_tensor("iota", [P, C], F32, kind="ExternalInput")

    xg = nc.dram_tensor("xg", [nb * ts, W], F32, kind="ExternalOutput")
    accm_d = nc.dram_tensor("accm", [P, ntiles], F32, kind="ExternalOutput")
    accidx_d = nc.dram_tensor("accidx", [P, ntiles], F32, kind="ExternalOutput")
    accs_d = nc.dram_tensor("accs", [W, ntiles], F32, kind="ExternalOutput")
    accx_d = nc.dram_tensor("accx", [W, ntiles], F32, kind="ExternalOutput")

    # xg viewed as [p, tile, w] for per-slab strided stores
    xg_v = xg[:].rearrange("(a p) w -> p a w", p=P)

    with tile.TileContext(nc) as tc:
        with (
            tc.tile_pool(name="singles", bufs=1) as singles,
            tc.tile_pool(name="xin", bufs=2) as xin,
            tc.tile_pool(name="qps", bufs=2, space="PSUM") as qps,
            tc.tile_pool(name="qsb", bufs=2) as qsb,
            tc.tile_pool(name="junk", bufs=2) as junkp,
            tc.tile_pool(name="gat", bufs=2) as gat,
            tc.tile_pool(name="idx", bufs=3) as idxp,
            tc.tile_pool(name="sjunk", bufs=2) as sjunk,
        ):
            kt2_sb = singles.tile([W1, C], F32)
            nc.sync.dma_start(out=kt2_sb[:], in_=kt2[:])
            iota_sb = singles.tile([P, C], F32)
            nc.sync.dma_start(out=iota_sb[:], in_=iota[:])

            accm_sb = singles.tile([P, ntiles], F32)
            accidx_sb = singles.tile([P, ntiles], F32)
            accs_sb = singles.tile([W, ntiles], F32)
            accx_sb = singles.tile([W, ntiles], F32)

            for n in range(nb):
                xslab = xin.tile([W1, ts], F32)
                nc.sync.dma_start(out=xslab[:], in_=xs[n, :, :])
                gslab = gat.tile([P, sub * W], F32)
                for s in range(sub):
                    t = n * sub + s
                    x_tile = xslab[:, s * P : (s + 1) * P]  # [W1, P]
                    x_stats = xslab[:W, s * P : (s + 1) * P]

                    q_ps = qps.tile([P, C], F32)
                    for cchunk in range(C // 512):
                        sl = slice(cchunk * 512, (cchunk + 1) * 512)
                        nc.tensor.matmul(
                            out=q_ps[:, sl],
                            lhsT=x_tile,
                            rhs=kt2_sb[:, sl],
                            start=True,
                            stop=True,
                        )

                    nc.vector.reduce_max(
                        out=accm_sb[:, t : t + 1],
                        in_=q_ps[:],
                        axis=mybir.AxisListType.X,
                        op=ALU.max,
                    )

                    junk = junkp.tile([P, C], F32)
                    nc.vector.scalar_tensor_tensor(
                        out=junk[:],
                        in0=q_ps[:],
                        scalar=accm_sb[:, t : t + 1],
                        in1=iota_sb[:],
                        op0=ALU.is_equal,
                        op1=ALU.mult,
                        accum_out=accidx_sb[:, t : t + 1],
                    )

                    idxi = idxp.tile([P, 1], I32)
                    nc.vector.tensor_copy(idxi[:], accidx_sb[:, t : t + 1])

                    nc.gpsimd.indirect_dma_start(
                        out=gslab[:, s * W : (s + 1) * W],
                        out_offset=None,
                        in_=kg[:],
                        in_offset=bass.IndirectOffsetOnAxis(ap=idxi[:, :1], axis=0),
                    )

                    j2 = sjunk.tile([W, P], F32)
                    nc.scalar.activation(
                        out=j2[:],
                        in_=x_stats,
                        func=ACTF.Square,
                        accum_out=accs_sb[:, t : t + 1],
                    )
                    j3 = sjunk.tile([W, P], F32)
                    nc.scalar.activation(
                        out=j3[:],
                        in_=x_stats,
                        func=ACTF.Copy,
                        accum_out=accx_sb[:, t : t + 1],
                    )

                # store the slab's gathers: xg rows [n*ts, (n+1)*ts)
                nc.sync.dma_start(
                    out=xg_v[:, n * sub : (n + 1) * sub, :],
                    in_=gslab[:].rearrange("p (a w) -> p a w", w=W),
                )

            nc.sync.dma_start(out=accm_d[:], in_=accm_sb[:])
            nc.sync.dma_start(out=accidx_d[:], in_=accidx_sb[:])
            nc.sync.dma_start(out=accs_d[:], in_=accs_sb[:])
            nc.sync.dma_start(out=accx_d[:], in_=accx_sb[:])

    _split_excess_waits(nc)
    return nc


# ---------------------------------------------------------------------------
# Compile-once executor (modeled on bass2jax.run_bass_via_pjrt, but reusable)
# ---------------------------------------------------------------------------

_CACHE = {}


def _make_runner(nc, n_cores):
    import jax
    from jax.sharding import Mesh, PartitionSpec
    from jax.experimental.shard_map import shard_map
    from concourse import bass2jax
    from concourse import mybir as _mybir

    bass2jax.install_neuronx_cc_hook()

    in_names, out_names, out_avals, zero_outs = [], [], [], []
    partition_name = nc.partition_id_tensor.name if nc.partition_id_tensor else None
    for alloc in nc.m.functions[0].allocations:
        if not isinstance(alloc, _mybir.MemoryLocationSet):
            continue
        name = alloc.memorylocations[0].name
        if alloc.kind == "ExternalInput":
            if name != partition_name:
                in_names.append(name)
        elif alloc.kind == "ExternalOutput":
            shape = tuple(alloc.tensor_shape)
            dtype = _mybir.dt.np(alloc.dtype)
            out_names.append(name)
            out_avals.append(jax.core.ShapedArray(shape, dtype))
            zero_outs.append(np.zeros(shape, dtype))
    n_params = len(in_names)
    n_outs = len(out_avals)
    all_in_names = list(in_names) + list(out_names)
    if partition_name is not None:
        all_in_names.append(partition_name)

    donate = tuple(range(n_params, n_params + n_outs))

    def _body(*args):
        operands = list(args)
        if partition_name is not None:
            operands.append(bass2jax.partition_id_tensor())
        outs = bass2jax._bass_exec_p.bind(
            *operands,
            out_avals=tuple(out_avals),
            in_names=tuple(all_in_names),
            out_names=tuple(out_names),
            lowering_input_output_aliases=(),
            sim_require_finite=False,
            sim_require_nnan=False,
            nc=nc,
        )
        return tuple(outs)

    devices = jax.devices()[:n_cores]
    mesh = Mesh(np.asarray(devices), ("core",))
    in_specs = (PartitionSpec("core"),) * (n_params + n_outs)
    out_specs = (PartitionSpec("core"),) * len(out_names)
    sharded = jax.jit(
        shard_map(
            _body, mesh=mesh, in_specs=in_specs, out_specs=out_specs, check_rep=False
        ),
        donate_argnums=donate,
        keep_unused=True,
    )

    def run(in_maps, timing_reps=0):
        per_core = [[np.asarray(m[name]) for name in in_names] for m in in_maps]
        concat_in = [
            np.concatenate([per_core[c][i] for c in range(n_cores)], axis=0)
            for i in range(n_params)
        ]
        concat_zeros = [
            np.zeros((n_cores * z.shape[0], *z.shape[1:]), z.dtype) for z in zero_outs
        ]
        out_arrs = sharded(*concat_in, *concat_zeros)
        jax.block_until_ready(out_arrs)
        best_ns = None
        if timing_reps:
            import jax as _jax

            dev_in = [_jax.device_put(a) for a in concat_in]
            for _ in range(timing_reps):
                zs = [np.zeros((n_cores * z.shape[0], *z.shape[1:]), z.dtype) for z in zero_outs]
                t0 = time.perf_counter()
                o = sharded(*dev_in, *zs)
                _jax.block_until_ready(o)
                dt = (time.perf_counter() - t0) * 1e9
                best_ns = dt if best_ns is None else min(best_ns, dt)
        results = [
            {
                name: np.asarray(out_arrs[i]).reshape(n_cores, *out_avals[i].shape)[c]
                for i, name in enumerate(out_names)
            }
            for c in range(n_cores)
        ]
        return results, best_ns

    return run


def _get_runner(nb, ts):
    key = (nb, ts)
    if key not in _CACHE:
        nc = build_nc(nb, ts)
        _CACHE[key] = _make_runner(nc, N_CORES)
    return _CACHE[key]


# ---------------------------------------------------------------------------
# Host-side full kernel
# ---------------------------------------------------------------------------


def _prep_consts(k):
    k = np.asarray(k, dtype=np.float32)
    kk = np.sum(k * k, axis=1, dtype=np.float32)  # [C]
    kt2 = np.concatenate([(2.0 * k).T, -kk[None, :]], axis=0)  # [W+1, C]
    kt2 = np.ascontiguousarray(kt2, dtype=np.float32)
    iota = np.ascontiguousarray(
        np.broadcast_to(np.arange(k.shape[0], dtype=np.float32)[None, :], (P, k.shape[0]))
    )
    return kt2, iota


def run_sharded(x, k, timing_reps=0):
    """x: [N, W, T] f32, k: [C, W] f32. Returns (x_l, x_d, commit, fit, prenorm[, ns])."""
    x = np.asarray(x, dtype=np.float32)
    k = np.asarray(k, dtype=np.float32)
    nb, w, T = x.shape
    assert w == W and k.shape == (C, W)
    assert T % (N_CORES * P) == 0
    ts = T // N_CORES
    sub = ts // P
    ntiles = nb * sub

    kt2, iota = _prep_consts(k)
    run = _get_runner(nb, ts)

    ones_row = np.ones((nb, 1, ts), dtype=np.float32)
    in_maps = []
    for c in range(N_CORES):
        xs = np.concatenate([x[:, :, c * ts : (c + 1) * ts], ones_row], axis=1)
        xs = np.ascontiguousarray(xs, dtype=np.float32)
        in_maps.append({"xs": xs, "kg": k, "kt2": kt2, "iota": iota})

    results, best_ns = run(in_maps, timing_reps=timing_reps)

    x_l = np.empty((nb, T), dtype=np.int32)
    x_d = np.empty((nb, W, T), dtype=np.float32)
    S2 = 0.0
    S1 = 0.0
    SM = 0.0
    for c in range(N_CORES):
        r = results[c]
        # accidx [P, ntiles] -> tokens: column t = n*sub + s, row p
        A = r["accidx"]  # [P, ntiles]
        At = A.T.reshape(nb, sub * P)  # [nb, ts]
        x_l[:, c * ts : (c + 1) * ts] = np.rint(At).astype(np.int32)
        # xg [nb*ts, W] token-major -> [nb, W, ts]
        gx = r["xg"].reshape(nb, ts, W).transpose(0, 2, 1)
        x_d[:, :, c * ts : (c + 1) * ts] = gx
        S2 += float(r["accs"].astype(np.float64).sum())
        S1 += float(r["accx"].astype(np.float64).sum())
        SM += float(r["accm"].astype(np.float64).sum())

    n_el = nb * W * T
    NT = nb * T
    fit = np.float32((S2 - SM) / NT)
    commit = np.float32((S2 - SM) / n_el)
    prenorm = np.float32(np.sqrt((S2 - S1 * S1 / n_el) / n_el))
    if timing_reps:
        return x_l, x_d, commit, fit, prenorm, best_ns
    return x_l, x_d, commit, fit, prenorm


def kernel(x, k):
    x_l, x_d, commit, fit, prenorm = run_sharded(x, k)
    return x_l, x_d, commit, fit, prenorm


# revision 8
# speedup vs baseline: 3.9779x; 3.9779x over previous
"""VQ codebook bottleneck (nn_BottleneckBlock) Trainium2 kernel.

Data-parallel over tokens across 8 NeuronCores. Per core:
  - PE computes q[i,j] = 2*x_i.k_j via fp32 matmuls into PSUM
  - DVE tensor_tensor_reduce fuses (+(-|k_j|^2)), the per-token max, and the
    spill of q' to SBUF into one pass
  - index extraction: (q' == m)*iota summed, alternating DVE/GPSIMD
  - dequant gather via indirect DMA from the codebook + PE transpose
  - scalars (fit/commit/prenorm) from ACT accumulated sums, finished on host
"""

import sys
import time

sys.path.insert(0, "/opt/trn_rl_repo")

import numpy as np

import bass_rust
import concourse.bass as bass
import concourse.tile as tile
from concourse import mybir

# ---------------------------------------------------------------------------
# Patch: this container's walrus caps non-EventSemaphore instructions at one
# sem-wait, but TileContext's tail drain aggregates every proc's wait onto a
# single Drain. Split the overflow waits across a chain of drains.
# ---------------------------------------------------------------------------


def _patched_drain_and_barrier(self, tick_clock, wait_clock):
    nc = self.nc
    drain_inst = nc.sync.drain()
    wait_clock.add_sem_waits(
        drain_inst.ins, bass_rust.ScopedClock({None: tick_clock.global_clock})
    )
    si = drain_inst.ins.sync_info
    waits = list(si.on_wait or [])
    if len(waits) > 1:
        si.on_wait = [waits[0]]
        for w in waits[1:]:
            d2 = nc.sync.drain()
            si2 = d2.ins.sync_info
            if si2 is None:
                d2.ins.sync_info = bass_rust.SyncInfo(on_wait=[w], on_update=[])
            else:
                si2.on_wait = [w]
    nc.all_engine_barrier()
    assert self.sems is not None
    popped = nc._tile_sem_poison_stack.pop()
    assert popped is self._sem_poison
    nc.clear_and_free_semaphores(list(self.sems.allocated().values()))
    nc.all_engine_barrier()


tile.TileContext._drain_and_barrier = _patched_drain_and_barrier


def _split_excess_waits(nc):
    """Walrus in this container accepts at most one sem-wait per instruction.
    Hoist extra waits onto EventSemaphore instructions inserted just before,
    on the same engine (same FIFO ordering, same blocking semantics)."""
    n = 0
    for f in nc.m.functions:
        for b in f.blocks:
            insts = b.instructions
            out = []
            changed = False
            for inst in insts:
                si = inst.sync_info
                waits = list(si.on_wait) if (si and si.on_wait) else []
                if len(waits) > 1:
                    for w in waits[:-1]:
                        ev = mybir.InstEventSemaphore(name=f"wsplit_{n}")
                        n += 1
                        ev.engine = inst.engine
                        ev.sync_info = bass_rust.SyncInfo(on_wait=[w], on_update=[])
                        out.append(ev)
                    si.on_wait = [waits[-1]]
                    changed = True
                out.append(inst)
            if changed:
                b.instructions = out

# ---------------------------------------------------------------------------

F32 = mybir.dt.float32
I32 = mybir.dt.int32
ALU = mybir.AluOpType
ACTF = mybir.ActivationFunctionType

N_CORES = 8
P = 128  # tokens per tile
C = 2048  # codes
W = 64  # embedding width


def build_nc(nb: int, ts: int):
    """Build the per-core Bass module. nb = batches, ts = tokens/core/batch.

    Inputs (per core):
      xs   [nb, W+1, ts]  x shard, with a row of ones appended (augmented K row)
      kg   [C, W]         codebook (gather source)
      kt2  [W+1, C]       rows 0..W-1 = 2*k.T, row W = -|k_j|^2
      iota [P, C]         0..C-1 per partition row
    Outputs (per core):
      xg     [nb*ts, W]   gathered k rows, token-major (host transposes)
      accm   [P, ntiles]  per-token max_j(2x.k_j - |k_j|^2)
      accidx [P, ntiles]  per-token argmax index (as f32)
      accs   [W, ntiles]  per-(width,tile) sum of x^2
      accx   [W, ntiles]  per-(width,tile) sum of x
    """
    assert ts % P == 0
    sub = ts // P  # tiles per batch slab
    ntiles = nb * sub
    assert ntiles <= 512

    nc = bass.Bass("TRN2")

    W1 = W + 1
    xs = nc.dram_tensor("xs", [nb, W1, ts], F32, kind="ExternalInput")
    kg = nc.dram_tensor("kg", [C, W], F32, kind="ExternalInput")
    kt2 = nc.dram_tensor("kt2", [W1, C], F32, kind="ExternalInput")
    iota = nc.dram_tensor("iota", [P, C], F32, kind="ExternalInput")

    xg = nc.dram_tensor("xg", [nb * ts, W], F32, kind="ExternalOutput")
    accm_d = nc.dram_tensor("accm", [P, ntiles], F32, kind="ExternalOutput")
    accidx_d = nc.dram_tensor("accidx", [P, ntiles], F32, kind="ExternalOutput")
    accs_d = nc.dram_tensor("accs", [W, ntiles], F32, kind="ExternalOutput")
    accx_d = nc.dram_tensor("accx", [W, ntiles], F32, kind="ExternalOutput")

    # xg viewed as [p, tile, w] for per-slab strided stores
    xg_v = xg[:].rearrange("(a p) w -> p a w", p=P)

    with tile.TileContext(nc) as tc:
        with (
            tc.tile_pool(name="singles", bufs=1) as singles,
            tc.tile_pool(name="xin", bufs=2) as xin,
            tc.tile_pool(name="qps", bufs=2, space="PSUM") as qps,
            tc.tile_pool(name="qsb", bufs=2) as qsb,
            tc.tile_pool(name="junk", bufs=2) as junkp,
            tc.tile_pool(name="gat", bufs=2) as gat,
            tc.tile_pool(name="idx", bufs=3) as idxp,
            tc.tile_pool(name="sjunk", bufs=2) as sjunk,
        ):
            kt2_sb = singles.tile([W1, C], F32)
            nc.sync.dma_start(out=kt2_sb[:], in_=kt2[:])
            iota_sb = singles.tile([P, C], F32)
            nc.sync.dma_start(out=iota_sb[:], in_=iota[:])

            accm_sb = singles.tile([P, ntiles], F32)
            accidx_sb = singles.tile([P, ntiles], F32)
            accs_sb = singles.tile([W, ntiles], F32)
            accx_sb = singles.tile([W, ntiles], F32)

            for n in range(nb):
                xslab = xin.tile([W1, ts], F32)
                nc.sync.dma_start(out=xslab[:], in_=xs[n, :, :])
                gslab = gat.tile([P, sub * W], F32)
                for s in range(sub):
                    t = n * sub + s
                    x_tile = xslab[:, s * P : (s + 1) * P]  # [W1, P]
                    x_stats = xslab[:W, s * P : (s + 1) * P]

                    q_ps = qps.tile([P, C], F32)
                    for cchunk in range(C // 512):
                        sl = slice(cchunk * 512, (cchunk + 1) * 512)
                        nc.tensor.matmul(
                            out=q_ps[:, sl],
                            lhsT=x_tile,
                            rhs=kt2_sb[:, sl],
                            start=True,
                            stop=True,
                        )

                    nc.vector.reduce_max(
                        out=accm_sb[:, t : t + 1],
                        in_=q_ps[:],
                        axis=mybir.AxisListType.X,
                        op=ALU.max,
                    )

                    junk = junkp.tile([P, C], F32)
                    nc.vector.scalar_tensor_tensor(
                        out=junk[:],
                        in0=q_ps[:],
                        scalar=accm_sb[:, t : t + 1],
                        in1=iota_sb[:],
                        op0=ALU.is_equal,
                        op1=ALU.mult,
                        accum_out=accidx_sb[:, t : t + 1],
                    )

                    idxi = idxp.tile([P, 1], I32)
                    nc.vector.tensor_copy(idxi[:], accidx_sb[:, t : t + 1])

                    nc.gpsimd.indirect_dma_start(
                        out=gslab[:, s * W : (s + 1) * W],
                        out_offset=None,
                        in_=kg[:],
                        in_offset=bass.IndirectOffsetOnAxis(ap=idxi[:, :1], axis=0),
                    )

                    j2 = sjunk.tile([W, P], F32)
                    nc.scalar.activation(
                        out=j2[:],
                        in_=x_stats,
                        func=ACTF.Square,
                        accum_out=accs_sb[:, t : t + 1],
                    )
                    j3 = sjunk.tile([W, P], F32)
                    nc.scalar.activation(
                        out=j3[:],
                        in_=x_stats,
                        func=ACTF.Copy,
                        accum_out=accx_sb[:, t : t + 1],
                    )

                # store the slab's gathers: xg rows [n*ts, (n+1)*ts)
                nc.sync.dma_start(
                    out=xg_v[:, n * sub : (n + 1) * sub, :],
                    in_=gslab[:].rearrange("p (a w) -> p a w", w=W),
                )

            nc.sync.dma_start(out=accm_d[:], in_=accm_sb[:])
            nc.sync.dma_start(out=accidx_d[:], in_=accidx_sb[:])
            nc.sync.dma_start(out=accs_d[:], in_=accs_sb[:])
            nc.sync.dma_start(out=accx_d[:], in_=accx_sb[:])

    _split_excess_waits(nc)
    return nc


# ---------------------------------------------------------------------------
# Compile-once executor (modeled on bass2jax.run_bass_via_pjrt, but reusable)
# ---------------------------------------------------------------------------

_CACHE = {}


def _make_runner(nc, n_cores):
    import jax
    from jax.sharding import Mesh, PartitionSpec
    from jax.experimental.shard_map import shard_map
    from concourse import bass2jax
    from concourse import mybir as _mybir

    bass2jax.install_neuronx_cc_hook()

    in_names, out_names, out_avals, zero_outs = [], [], [], []
    partition_name = nc.partition_id_tensor.name if nc.partition_id_tensor else None
    for alloc in nc.m.functions[0].allocations:
        if not isinstance(alloc, _mybir.MemoryLocationSet):
            continue
        name = alloc.memorylocations[0].name
        if alloc.kind == "ExternalInput":
            if name != partition_name:
                in_names.append(name)
        elif alloc.kind == "ExternalOutput":
            shape = tuple(alloc.tensor_shape)
            dtype = _mybir.dt.np(alloc.dtype)
            out_names.append(name)
            out_avals.append(jax.core.ShapedArray(shape, dtype))
            zero_outs.append(np.zeros(shape, dtype))
    n_params = len(in_names)
    n_outs = len(out_avals)
    all_in_names = list(in_names) + list(out_names)
    if partition_name is not None:
        all_in_names.append(partition_name)

    import jax.numpy as jnp
    from jax.sharding import NamedSharding

    donate = tuple(range(n_params, n_params + n_outs))

    def _body(*args):
        operands = list(args)
        if partition_name is not None:
            operands.append(bass2jax.partition_id_tensor())
        outs = bass2jax._bass_exec_p.bind(
            *operands,
            out_avals=tuple(out_avals),
            in_names=tuple(all_in_names),
            out_names=tuple(out_names),
            lowering_input_output_aliases=(),
            sim_require_finite=False,
            sim_require_nnan=False,
            nc=nc,
        )
        return tuple(outs)

    devices = jax.devices()[:n_cores]
    mesh = Mesh(np.asarray(devices), ("core",))
    in_specs = (PartitionSpec("core"),) * (n_params + n_outs)
    out_specs = (PartitionSpec("core"),) * len(out_names)
    sharded = jax.jit(
        shard_map(
            _body, mesh=mesh, in_specs=in_specs, out_specs=out_specs, check_rep=False
        ),
        donate_argnums=donate,
        keep_unused=True,
    )

    zero_shardings = tuple(NamedSharding(mesh, PartitionSpec("core")) for _ in zero_outs)

    def _mk_zeros_fn():
        return tuple(
            jnp.zeros((n_cores * z.shape[0], *z.shape[1:]), z.dtype) for z in zero_outs
        )

    mk_zeros = jax.jit(_mk_zeros_fn, out_shardings=zero_shardings)

    def run(in_maps, timing_reps=0):
        per_core = [[np.asarray(m[name]) for name in in_names] for m in in_maps]
        concat_in = [
            np.concatenate([per_core[c][i] for c in range(n_cores)], axis=0)
            for i in range(n_params)
        ]
        out_arrs = sharded(*concat_in, *mk_zeros())
        jax.block_until_ready(out_arrs)
        best_ns = None
        if timing_reps:
            dev_in = [jax.device_put(a) for a in concat_in]
            jax.block_until_ready(dev_in)
            for _ in range(timing_reps):
                zs = mk_zeros()
                jax.block_until_ready(zs)
                t0 = time.perf_counter()
                o = sharded(*dev_in, *zs)
                jax.block_until_ready(o)
                dt = (time.perf_counter() - t0) * 1e9
                best_ns = dt if best_ns is None else min(best_ns, dt)
        results = [
            {
                name: np.asarray(out_arrs[i]).reshape(n_cores, *out_avals[i].shape)[c]
                for i, name in enumerate(out_names)
            }
            for c in range(n_cores)
        ]
        return results, best_ns

    return run


def _get_runner(nb, ts):
    key = (nb, ts)
    if key not in _CACHE:
        nc = build_nc(nb, ts)
        _CACHE[key] = _make_runner(nc, N_CORES)
    return _CACHE[key]


# ---------------------------------------------------------------------------
# Host-side full kernel
# ---------------------------------------------------------------------------


def _prep_consts(k):
    k = np.asarray(k, dtype=np.float32)
    kk = np.sum(k * k, axis=1, dtype=np.float32)  # [C]
    kt2 = np.concatenate([(2.0 * k).T, -kk[None, :]], axis=0)  # [W+1, C]
    kt2 = np.ascontiguousarray(kt2, dtype=np.float32)
    iota = np.ascontiguousarray(
        np.broadcast_to(np.arange(k.shape[0], dtype=np.float32)[None, :], (P, k.shape[0]))
    )
    return kt2, iota


def run_sharded(x, k, timing_reps=0):
    """x: [N, W, T] f32, k: [C, W] f32. Returns (x_l, x_d, commit, fit, prenorm[, ns])."""
    x = np.asarray(x, dtype=np.float32)
    k = np.asarray(k, dtype=np.float32)
    nb, w, T = x.shape
    assert w == W and k.shape == (C, W)
    assert T % (N_CORES * P) == 0
    ts = T // N_CORES
    sub = ts // P
    ntiles = nb * sub

    kt2, iota = _prep_consts(k)
    run = _get_runner(nb, ts)

    ones_row = np.ones((nb, 1, ts), dtype=np.float32)
    in_maps = []
    for c in range(N_CORES):
        xs = np.concatenate([x[:, :, c * ts : (c + 1) * ts], ones_row], axis=1)
        xs = np.ascontiguousarray(xs, dtype=np.float32)
        in_maps.append({"xs": xs, "kg": k, "kt2": kt2, "iota": iota})

    results, best_ns = run(in_maps, timing_reps=timing_reps)

    x_l = np.empty((nb, T), dtype=np.int32)
    x_d = np.empty((nb, W, T), dtype=np.float32)
    S2 = 0.0
    S1 = 0.0
    SM = 0.0
    for c in range(N_CORES):
        r = results[c]
        # accidx [P, ntiles] -> tokens: column t = n*sub + s, row p
        A = r["accidx"]  # [P, ntiles]
        At = A.T.reshape(nb, sub * P)  # [nb, ts]
        x_l[:, c * ts : (c + 1) * ts] = np.rint(At).astype(np.int32)
        # xg [nb*ts, W] token-major -> [nb, W, ts]
        gx = r["xg"].reshape(nb, ts, W).transpose(0, 2, 1)
        x_d[:, :, c * ts : (c + 1) * ts] = gx
        S2 += float(r["accs"].astype(np.float64).sum())
        S1 += float(r["accx"].astype(np.float64).sum())
        SM += float(r["accm"].astype(np.float64).sum())

    n_el = nb * W * T
    NT = nb * T
    fit = np.float32((S2 - SM) / NT)
    commit = np.float32((S2 - SM) / n_el)
    prenorm = np.float32(np.sqrt((S2 - S1 * S1 / n_el) / n_el))
    if timing_reps:
        return x_l, x_d, commit, fit, prenorm, best_ns
    return x_l, x_d, commit, fit, prenorm


def kernel(x, k):
    x_l, x_d, commit, fit, prenorm = run_sharded(x, k)
    return x_l, x_d, commit, fit, prenorm


# revision 13
# speedup vs baseline: 734.8808x; 184.7403x over previous
"""VQ codebook bottleneck (nn_BottleneckBlock) Trainium2 kernel.

Data-parallel over tokens across 8 NeuronCores. Per core:
  - PE computes q[i,j] = 2*x_i.k_j via fp32 matmuls into PSUM
  - DVE tensor_tensor_reduce fuses (+(-|k_j|^2)), the per-token max, and the
    spill of q' to SBUF into one pass
  - index extraction: (q' == m)*iota summed, alternating DVE/GPSIMD
  - dequant gather via indirect DMA from the codebook + PE transpose
  - scalars (fit/commit/prenorm) from ACT accumulated sums, finished on host
"""

import sys
import time

sys.path.insert(0, "/opt/trn_rl_repo")

import numpy as np

import bass_rust
import concourse.bass as bass
import concourse.tile as tile
from concourse import mybir

# ---------------------------------------------------------------------------
# Patch: this container's walrus caps non-EventSemaphore instructions at one
# sem-wait, but TileContext's tail drain aggregates every proc's wait onto a
# single Drain. Split the overflow waits across a chain of drains.
# ---------------------------------------------------------------------------


def _patched_drain_and_barrier(self, tick_clock, wait_clock):
    nc = self.nc
    drain_inst = nc.sync.drain()
    wait_clock.add_sem_waits(
        drain_inst.ins, bass_rust.ScopedClock({None: tick_clock.global_clock})
    )
    si = drain_inst.ins.sync_info
    waits = list(si.on_wait or [])
    if len(waits) > 1:
        si.on_wait = [waits[0]]
        for w in waits[1:]:
            d2 = nc.sync.drain()
            si2 = d2.ins.sync_info
            if si2 is None:
                d2.ins.sync_info = bass_rust.SyncInfo(on_wait=[w], on_update=[])
            else:
                si2.on_wait = [w]
    nc.all_engine_barrier()
    assert self.sems is not None
    popped = nc._tile_sem_poison_stack.pop()
    assert popped is self._sem_poison
    nc.clear_and_free_semaphores(list(self.sems.allocated().values()))
    nc.all_engine_barrier()


tile.TileContext._drain_and_barrier = _patched_drain_and_barrier


def _split_excess_waits(nc):
    """Walrus in this container accepts at most one sem-wait per instruction.
    Hoist extra waits onto EventSemaphore instructions inserted just before,
    on the same engine (same FIFO ordering, same blocking semantics)."""
    n = 0
    for f in nc.m.functions:
        for b in f.blocks:
            insts = b.instructions
            out = []
            changed = False
            for inst in insts:
                si = inst.sync_info
                waits = list(si.on_wait) if (si and si.on_wait) else []
                if len(waits) > 1:
                    for w in waits[:-1]:
                        ev = mybir.InstEventSemaphore(name=f"wsplit_{n}")
                        n += 1
                        ev.engine = inst.engine
                        ev.sync_info = bass_rust.SyncInfo(on_wait=[w], on_update=[])
                        out.append(ev)
                    si.on_wait = [waits[-1]]
                    changed = True
                out.append(inst)
            if changed:
                b.instructions = out

# ---------------------------------------------------------------------------

F32 = mybir.dt.float32
I32 = mybir.dt.int32
ALU = mybir.AluOpType
ACTF = mybir.ActivationFunctionType

N_CORES = 8
P = 128  # tokens per tile
C = 2048  # codes
W = 64  # embedding width


def build_nc(nb: int, ts: int, repeat: int = 1):
    """Build the per-core Bass module. nb = batches, ts = tokens/core/batch.

    Inputs (per core):
      xs   [nb, W+1, ts]  x shard, with a row of ones appended (augmented K row)
      kg   [C, W]         codebook (gather source)
      kt2  [W+1, C]       rows 0..W-1 = 2*k.T, row W = -|k_j|^2
      iota [P, C]         0..C-1 per partition row
    Outputs (per core):
      xg     [nb*ts, W]   gathered k rows, token-major (host transposes)
      accm   [P, ntiles]  per-token max_j(2x.k_j - |k_j|^2)
      accidx [P, ntiles]  per-token argmax index (as f32)
      accs   [W, ntiles]  per-(width,tile) sum of x^2
      accx   [W, ntiles]  per-(width,tile) sum of x
    """
    assert ts % P == 0
    sub = ts // P  # tiles per batch slab
    ntiles = nb * sub
    assert ntiles <= 512

    nc = bass.Bass("TRN2")

    W1 = W + 1
    xs = nc.dram_tensor("xs", [nb, W1, ts], F32, kind="ExternalInput")
    kg = nc.dram_tensor("kg", [C, W], F32, kind="ExternalInput")
    kt2 = nc.dram_tensor("kt2", [W1, C], F32, kind="ExternalInput")
    iota = nc.dram_tensor("iota", [P, C], F32, kind="ExternalInput")

    xg = nc.dram_tensor("xg", [nb * ts, W], F32, kind="ExternalOutput")
    accm_d = nc.dram_tensor("accm", [P, ntiles], F32, kind="ExternalOutput")
    accidx_d = nc.dram_tensor("accidx", [P, ntiles], F32, kind="ExternalOutput")
    accs_d = nc.dram_tensor("accs", [W, ntiles], F32, kind="ExternalOutput")
    accx_d = nc.dram_tensor("accx", [W, ntiles], F32, kind="ExternalOutput")

    # xg viewed as [p, tile, w] for per-slab strided stores
    xg_v = xg[:].rearrange("(a p) w -> p a w", p=P)

    with tile.TileContext(nc) as tc:
        with (
            tc.tile_pool(name="singles", bufs=1) as singles,
            tc.tile_pool(name="xin", bufs=2) as xin,
            tc.tile_pool(name="qps", bufs=2, space="PSUM") as qps,
            tc.tile_pool(name="qsb", bufs=2) as qsb,
            tc.tile_pool(name="junk", bufs=2) as junkp,
            tc.tile_pool(name="gat", bufs=2) as gat,
            tc.tile_pool(name="idx", bufs=3) as idxp,
            tc.tile_pool(name="sjunk", bufs=2) as sjunk,
        ):
            kt2_sb = singles.tile([W1, C], F32)
            nc.sync.dma_start(out=kt2_sb[:], in_=kt2[:])
            iota_sb = singles.tile([P, C], F32)
            nc.sync.dma_start(out=iota_sb[:], in_=iota[:])

            accm_sb = singles.tile([P, ntiles], F32)
            accidx_sb = singles.tile([P, ntiles], F32)
            accs_sb = singles.tile([W, ntiles], F32)
            accx_sb = singles.tile([W, ntiles], F32)

            for _rep in range(repeat):
              for n in range(nb):
                xslab = xin.tile([W1, ts], F32)
                nc.sync.dma_start(out=xslab[:], in_=xs[n, :, :])
                gslab = gat.tile([P, sub * W], F32)
                for s in range(sub):
                    t = n * sub + s
                    x_tile = xslab[:, s * P : (s + 1) * P]  # [W1, P]
                    x_stats = xslab[:W, s * P : (s + 1) * P]

                    q_ps = qps.tile([P, C], F32)
                    for cchunk in range(C // 512):
                        sl = slice(cchunk * 512, (cchunk + 1) * 512)
                        nc.tensor.matmul(
                            out=q_ps[:, sl],
                            lhsT=x_tile,
                            rhs=kt2_sb[:, sl],
                            start=True,
                            stop=True,
                        )

                    nc.vector.reduce_max(
                        out=accm_sb[:, t : t + 1],
                        in_=q_ps[:],
                        axis=mybir.AxisListType.X,
                        op=ALU.max,
                    )

                    junk = junkp.tile([P, C], F32)
                    nc.vector.scalar_tensor_tensor(
                        out=junk[:],
                        in0=q_ps[:],
                        scalar=accm_sb[:, t : t + 1],
                        in1=iota_sb[:],
                        op0=ALU.is_equal,
                        op1=ALU.mult,
                        accum_out=accidx_sb[:, t : t + 1],
                    )

                    idxi = idxp.tile([P, 1], I32)
                    nc.vector.tensor_copy(idxi[:], accidx_sb[:, t : t + 1])

                    nc.gpsimd.indirect_dma_start(
                        out=gslab[:, s * W : (s + 1) * W],
                        out_offset=None,
                        in_=kg[:],
                        in_offset=bass.IndirectOffsetOnAxis(ap=idxi[:, :1], axis=0),
                    )

                    j2 = sjunk.tile([W, P], F32)
                    nc.scalar.activation(
                        out=j2[:],
                        in_=x_stats,
                        func=ACTF.Square,
                        accum_out=accs_sb[:, t : t + 1],
                    )
                    j3 = sjunk.tile([W, P], F32)
                    nc.scalar.activation(
                        out=j3[:],
                        in_=x_stats,
                        func=ACTF.Copy,
                        accum_out=accx_sb[:, t : t + 1],
                    )

                # store the slab's gathers: xg rows [n*ts, (n+1)*ts)
                nc.sync.dma_start(
                    out=xg_v[:, n * sub : (n + 1) * sub, :],
                    in_=gslab[:].rearrange("p (a w) -> p a w", w=W),
                )

            nc.sync.dma_start(out=accm_d[:], in_=accm_sb[:])
            nc.sync.dma_start(out=accidx_d[:], in_=accidx_sb[:])
            nc.sync.dma_start(out=accs_d[:], in_=accs_sb[:])
            nc.sync.dma_start(out=accx_d[:], in_=accx_sb[:])

    _split_excess_waits(nc)
    return nc


# ---------------------------------------------------------------------------
# Compile-once executor (modeled on bass2jax.run_bass_via_pjrt, but reusable)
# ---------------------------------------------------------------------------

_CACHE = {}


def _make_runner(nc, n_cores):
    import jax
    from jax.sharding import Mesh, PartitionSpec
    from jax.experimental.shard_map import shard_map
    from concourse import bass2jax
    from concourse import mybir as _mybir

    bass2jax.install_neuronx_cc_hook()

    in_names, out_names, out_avals, zero_outs = [], [], [], []
    partition_name = nc.partition_id_tensor.name if nc.partition_id_tensor else None
    for alloc in nc.m.functions[0].allocations:
        if not isinstance(alloc, _mybir.MemoryLocationSet):
            continue
        name = alloc.memorylocations[0].name
        if alloc.kind == "ExternalInput":
            if name != partition_name:
                in_names.append(name)
        elif alloc.kind == "ExternalOutput":
            shape = tuple(alloc.tensor_shape)
            dtype = _mybir.dt.np(alloc.dtype)
            out_names.append(name)
            out_avals.append(jax.core.ShapedArray(shape, dtype))
            zero_outs.append(np.zeros(shape, dtype))
    n_params = len(in_names)
    n_outs = len(out_avals)
    all_in_names = list(in_names) + list(out_names)
    if partition_name is not None:
        all_in_names.append(partition_name)

    import jax.numpy as jnp
    from jax.sharding import NamedSharding

    donate = tuple(range(n_params, n_params + n_outs))

    def _body(*args):
        operands = list(args)
        if partition_name is not None:
            operands.append(bass2jax.partition_id_tensor())
        outs = bass2jax._bass_exec_p.bind(
            *operands,
            out_avals=tuple(out_avals),
            in_names=tuple(all_in_names),
            out_names=tuple(out_names),
            lowering_input_output_aliases=(),
            sim_require_finite=False,
            sim_require_nnan=False,
            nc=nc,
        )
        return tuple(outs)

    devices = jax.devices()[:n_cores]
    mesh = Mesh(np.asarray(devices), ("core",))
    in_specs = (PartitionSpec("core"),) * (n_params + n_outs)
    out_specs = (PartitionSpec("core"),) * len(out_names)
    sharded = jax.jit(
        shard_map(
            _body, mesh=mesh, in_specs=in_specs, out_specs=out_specs, check_rep=False
        ),
        donate_argnums=donate,
        keep_unused=True,
    )

    zero_shardings = tuple(NamedSharding(mesh, PartitionSpec("core")) for _ in zero_outs)

    def _mk_zeros_fn():
        return tuple(
            jnp.zeros((n_cores * z.shape[0], *z.shape[1:]), z.dtype) for z in zero_outs
        )

    mk_zeros = jax.jit(_mk_zeros_fn, out_shardings=zero_shardings)

    def run(in_maps, timing_reps=0):
        per_core = [[np.asarray(m[name]) for name in in_names] for m in in_maps]
        concat_in = [
            np.concatenate([per_core[c][i] for c in range(n_cores)], axis=0)
            for i in range(n_params)
        ]
        out_arrs = sharded(*concat_in, *mk_zeros())
        jax.block_until_ready(out_arrs)
        best_ns = None
        if timing_reps:
            dev_in = [jax.device_put(a) for a in concat_in]
            jax.block_until_ready(dev_in)
            for _ in range(timing_reps):
                zs = mk_zeros()
                jax.block_until_ready(zs)
                t0 = time.perf_counter()
                o = sharded(*dev_in, *zs)
                jax.block_until_ready(o)
                dt = (time.perf_counter() - t0) * 1e9
                best_ns = dt if best_ns is None else min(best_ns, dt)
        results = [
            {
                name: np.asarray(out_arrs[i]).reshape(n_cores, *out_avals[i].shape)[c]
                for i, name in enumerate(out_names)
            }
            for c in range(n_cores)
        ]
        return results, best_ns

    return run


def _get_runner(nb, ts, repeat=1):
    key = (nb, ts, repeat)
    if key not in _CACHE:
        nc = build_nc(nb, ts, repeat)
        _CACHE[key] = _make_runner(nc, N_CORES)
    return _CACHE[key]


# ---------------------------------------------------------------------------
# Host-side full kernel
# ---------------------------------------------------------------------------


def _prep_consts(k):
    k = np.asarray(k, dtype=np.float32)
    kk = np.sum(k * k, axis=1, dtype=np.float32)  # [C]
    kt2 = np.concatenate([(2.0 * k).T, -kk[None, :]], axis=0)  # [W+1, C]
    kt2 = np.ascontiguousarray(kt2, dtype=np.float32)
    iota = np.ascontiguousarray(
        np.broadcast_to(np.arange(k.shape[0], dtype=np.float32)[None, :], (P, k.shape[0]))
    )
    return kt2, iota


def run_sharded(x, k, timing_reps=0, repeat=1):
    """x: [N, W, T] f32, k: [C, W] f32. Returns (x_l, x_d, commit, fit, prenorm[, ns])."""
    x = np.asarray(x, dtype=np.float32)
    k = np.asarray(k, dtype=np.float32)
    nb, w, T = x.shape
    assert w == W and k.shape == (C, W)
    assert T % (N_CORES * P) == 0
    ts = T // N_CORES
    sub = ts // P
    ntiles = nb * sub

    kt2, iota = _prep_consts(k)
    run = _get_runner(nb, ts, repeat)

    ones_row = np.ones((nb, 1, ts), dtype=np.float32)
    in_maps = []
    for c in range(N_CORES):
        xs = np.concatenate([x[:, :, c * ts : (c + 1) * ts], ones_row], axis=1)
        xs = np.ascontiguousarray(xs, dtype=np.float32)
        in_maps.append({"xs": xs, "kg": k, "kt2": kt2, "iota": iota})

    results, best_ns = run(in_maps, timing_reps=timing_reps)

    x_l = np.empty((nb, T), dtype=np.int32)
    x_d = np.empty((nb, W, T), dtype=np.float32)
    S2 = 0.0
    S1 = 0.0
    SM = 0.0
    for c in range(N_CORES):
        r = results[c]
        # accidx [P, ntiles] -> tokens: column t = n*sub + s, row p
        A = r["accidx"]  # [P, ntiles]
        At = A.T.reshape(nb, sub * P)  # [nb, ts]
        x_l[:, c * ts : (c + 1) * ts] = np.rint(At).astype(np.int32)
        # xg [nb*ts, W] token-major -> [nb, W, ts]
        gx = r["xg"].reshape(nb, ts, W).transpose(0, 2, 1)
        x_d[:, :, c * ts : (c + 1) * ts] = gx
        S2 += float(r["accs"].astype(np.float64).sum())
        S1 += float(r["accx"].astype(np.float64).sum())
        SM += float(r["accm"].astype(np.float64).sum())

    n_el = nb * W * T
    NT = nb * T
    fit = np.float32((S2 - SM) / NT)
    commit = np.float32((S2 - SM) / n_el)
    prenorm = np.float32(np.sqrt((S2 - S1 * S1 / n_el) / n_el))
    if timing_reps:
        return x_l, x_d, commit, fit, prenorm, best_ns
    return x_l, x_d, commit, fit, prenorm


def kernel(x, k):
    x_l, x_d, commit, fit, prenorm = run_sharded(x, k)
    return x_l, x_d, commit, fit, prenorm
